# revision 1
# baseline (speedup 1.0000x reference)
"""FALCON ObjectSomeValuesFrom forward kernel for Trainium2 (Bass/Tile).

Math: the reference computes
    c_fs[j]   = sigmoid(cw + col_j + b)
    r_fs[i,j] = sigmoid(row_i + col_j + b)
    out[i]    = max_j r_fs[i,j] * c_fs[j]
with col_j = e_j . w_r, row_i = e_i . w_l + rw, cw = c_emb . w_l,
rw = r_emb . w_l.  Both product factors are strictly increasing in col_j,
so the max over j is attained at argmax_j col_j for every i:
    out[i] = sigmoid(a_i + rw + colmax + b) * sigmoid(cw + colmax + b)
with a_i = e_i . w_l and colmax = max_j col_j.  The O(N^2) pairwise block
collapses to two GEMVs over e_all plus an elementwise sigmoid tail.

Sharding: rows are split across the 8 cores.  Each core redundantly
computes colmax over the full table (4 MB read, chunk-pipelined DMA+DVE)
and the w_l GEMV + sigmoid tail for its own 1024-row slice.  No
cross-core communication.
"""

import numpy as np

N = 8192        # 8000 named + 192 anon entities
D = 128         # emb dim
NCORES = 8
RPC = N // NCORES    # rows per core (1024)
P = 128              # SBUF partitions
NPC_FULL = N // P    # 64 rows of e_all per partition
NPC_ROWS = RPC // P  # 8 rows of the core slice per partition
CHUNK = 16           # rows-per-partition per pipeline chunk
NCHUNK = NPC_FULL // CHUNK  # 8 chunks over the full table
ACT_RED_CHUNKS = 2   # col-scan chunks whose reduce runs on ACT (rest on DVE)
GP_ROW_MUL = False   # row-pass multiply on GPSIMD, reduce on ACT
EIN_BUFS = 3
EPROD_BUFS = 3
ROW_RED_ACT = False  # row-pass reduce on ACT even when the mul stays on DVE
GP_PAIR_CHUNKS = 0   # chunks whose product gets a GPSIMD pairwise-add halving
GP_MUL_CHUNKS = 0    # trailing chunks whose multiply runs on GPSIMD
GP_MUL_FIRST = False # multiply of the (ACT-reduced) first chunk on GPSIMD
ROW_LAST = True      # trace the row pass after the col scan
CHUNK_SCHED = [16, 4, 16, 28]  # chunk sizes; ACT reduces chunks 0-1
ROW_DT_SAME = True   # row pass in the scan dtype (fp16) instead of exact f32
DMA_SPLIT = 1        # dma_starts per scan chunk (parallel queues)

_CACHE = {}
COL_DT = "fp16"  # colmax-scan precision: "f32" (exact), "fp16", or "bf16"


def _build_nc(repeat=1, col_dt="f32"):
    import concourse.bass as bass
    import concourse.bacc as bacc
    import concourse.tile as tile
    import concourse.mybir as mybir
    from concourse import bass_isa

    f32 = mybir.dt.float32
    cdt = {"f32": f32, "bf16": mybir.dt.bfloat16, "fp16": mybir.dt.float16}[col_dt]
    nc = bacc.Bacc("TRN2", target_bir_lowering=False, debug=False)

    e_full = nc.dram_tensor("e_full", [N, D], cdt, kind="ExternalInput").ap()
    rdt = cdt if (ROW_DT_SAME and col_dt != "f32") else f32
    e_rows = nc.dram_tensor("e_rows", [RPC, D], rdt, kind="ExternalInput").ap()
    wb = nc.dram_tensor("wb", [P, 2 * D], f32, kind="ExternalInput").ap()
    wr_c = nc.dram_tensor("wr_c", [P, 2 * D], cdt, kind="ExternalInput").ap()
    consts = nc.dram_tensor("consts", [P, 2], f32, kind="ExternalInput").ap()
    out = nc.dram_tensor("out", [RPC], f32, kind="ExternalOutput").ap()

    ev3 = e_full.rearrange("(p n) k -> p n k", p=P)  # [128, 64, 128]

    with tile.TileContext(nc) as tc:
        with (
            tc.tile_pool(name="ein", bufs=EIN_BUFS) as ein,
            tc.tile_pool(name="eprod", bufs=EPROD_BUFS) as eprod,
            tc.tile_pool(name="sb", bufs=1) as sb,
            tc.tile_pool(name="acc", bufs=2) as acc,
        ):
            wb_t = sb.tile([P, 2 * D], f32)
            nc.sync.dma_start(wb_t[:], wb)
            consts_t = sb.tile([P, 2], f32)
            nc.sync.dma_start(consts_t[:], consts)
            wrc_t = sb.tile([P, 2 * D], cdt)
            nc.sync.dma_start(wrc_t[:], wr_c)

            def wr_bcast(count):
                a = wrc_t[:, D : 2 * D]
                return bass.AP(a.tensor, a.offset, [a.ap[0], [0, count], a.ap[1]])

            def wl_bcast(count):
                a = wrc_t[:, 0:D] if rdt != f32 else wb_t[:, 0:D]
                return bass.AP(a.tensor, a.offset, [a.ap[0], [0, count], a.ap[1]])

            # Row slice pass: a_i = e_i . w_l for this core's rows.
            av = sb.tile([P, NPC_ROWS], f32)

            def row_pass():
                er_t = sb.tile([P, NPC_ROWS * D], rdt, name="er_t")
                er3 = er_t[:].rearrange("p (n k) -> p n k", k=D)
                erv3 = e_rows.rearrange("(p n) k -> p n k", p=P)
                nc.sync.dma_start(er3[:, :, :], erv3)
                rowp = sb.tile([P, NPC_ROWS * D], rdt, name="rowp")
                rowp3 = rowp[:].rearrange("p (n k) -> p n k", k=D)
                row_mul_eng = nc.gpsimd if GP_ROW_MUL else nc.vector
                row_mul_eng.tensor_tensor(
                    rowp3, er3, wl_bcast(NPC_ROWS), op=mybir.AluOpType.mult
                )
                if GP_ROW_MUL or ROW_RED_ACT:
                    rscratch = sb.tile([P, D], f32, name="rscratch")
                    for n in range(NPC_ROWS):
                        nc.scalar.activation(
                            rscratch[:],
                            rowp3[:, n, :],
                            mybir.ActivationFunctionType.Identity,
                            accum_out=av[:, n : n + 1],
                        )
                else:
                    nc.vector.reduce_sum(av[:], rowp3, axis=mybir.AxisListType.X)

            if not ROW_LAST:
                row_pass()

            # Full-table scan (repeat times for benchmarking; repeat=1 in
            # production): chunk-pipelined DMA -> mul -> per-chunk reduce.
            sched = CHUNK_SCHED or [CHUNK] * NCHUNK
            assert sum(sched) == NPC_FULL
            ncnk = len(sched)
            starts = [sum(sched[:i]) for i in range(ncnk)]
            colm_run = None
            for r in range(repeat):
                colv = acc.tile([P, NPC_FULL], f32, tag="colv")
                for c in range(ncnk):
                    cs, cn = starts[c], sched[c]
                    et = ein.tile([P, cn * D], cdt, tag=f"echunk{c}")
                    et3 = et[:].rearrange("p (n k) -> p n k", k=D)
                    dsp = DMA_SPLIT if cn % DMA_SPLIT == 0 else 1
                    dstep = cn // dsp
                    for d in range(dsp):
                        nc.sync.dma_start(
                            et3[:, d * dstep : (d + 1) * dstep, :],
                            ev3[:, cs + d * dstep : cs + (d + 1) * dstep, :],
                        )
                    pt = eprod.tile([P, cn * D], cdt, tag=f"pchunk{c}")
                    pt3 = pt[:].rearrange("p (n k) -> p n k", k=D)
                    mul_eng = (
                        nc.gpsimd
                        if (c >= ncnk - GP_MUL_CHUNKS or (GP_MUL_FIRST and c == 0))
                        else nc.vector
                    )
                    mul_eng.tensor_tensor(
                        pt3, et3, wr_bcast(cn), op=mybir.AluOpType.mult
                    )
                    if c < ACT_RED_CHUNKS:
                        ascr = eprod.tile([P, D], cdt, tag="ascratch")
                        for n in range(cn):
                            nc.scalar.activation(
                                ascr[:],
                                pt3[:, n, :],
                                mybir.ActivationFunctionType.Identity,
                                accum_out=colv[:, cs + n : cs + n + 1],
                            )
                    elif c < ACT_RED_CHUNKS + GP_PAIR_CHUNKS:
                        hp = eprod.tile([P, cn * D // 2], f32, tag="hchunk")
                        hp3 = hp[:].rearrange("p (n k) -> p n k", k=D // 2)
                        nc.gpsimd.tensor_tensor(
                            hp3,
                            pt3[:, :, 0 : D // 2],
                            pt3[:, :, D // 2 : D],
                            op=mybir.AluOpType.add,
                        )
                        nc.vector.reduce_sum(
                            colv[:, cs : cs + cn], hp3, axis=mybir.AxisListType.X
                        )
                    else:
                        nc.vector.reduce_sum(
                            colv[:, cs : cs + cn], pt3, axis=mybir.AxisListType.X
                        )
                colm = acc.tile([P, 1], f32, tag="colm")
                nc.vector.reduce_max(colm[:], colv[:], axis=mybir.AxisListType.X)
                if colm_run is None:
                    colm_run = colm
                else:
                    prev = colm_run
                    colm_run = acc.tile([P, 1], f32, tag="colmrun")
                    nc.vector.tensor_tensor(
                        colm_run[:], prev[:], colm[:], op=mybir.AluOpType.max
                    )

            if ROW_LAST:
                row_pass()

            colmax = sb.tile([P, 1], f32)
            nc.gpsimd.partition_all_reduce(
                colmax[:], colm_run[:], channels=P, reduce_op=bass_isa.ReduceOp.max
            )

            # K1 = colmax + (rw + b);  K2 = sigmoid(colmax + (cw + b))
            k1 = sb.tile([P, 1], f32)
            nc.vector.tensor_tensor(
                k1[:], colmax[:], consts_t[:, 0:1], op=mybir.AluOpType.add
            )
            k2p = sb.tile([P, 1], f32)
            nc.vector.tensor_tensor(
                k2p[:], colmax[:], consts_t[:, 1:2], op=mybir.AluOpType.add
            )
            k2 = sb.tile([P, 1], f32)
            nc.scalar.activation(k2[:], k2p[:], mybir.ActivationFunctionType.Sigmoid)

            # out = sigmoid(a + K1) * K2
            so = sb.tile([P, NPC_ROWS], f32)
            nc.scalar.activation(
                so[:],
                av[:],
                mybir.ActivationFunctionType.Sigmoid,
                bias=k1[:, 0:1],
            )
            fo = sb.tile([P, NPC_ROWS], f32)
            nc.vector.tensor_scalar_mul(fo[:], so[:], k2[:, 0:1])

            outv = out.rearrange("(p n) -> p n", p=P)
            nc.sync.dma_start(outv, fo[:])

    nc.compile()
    return nc


def get_nc(repeat=1, col_dt="f32"):
    key = ("nc", repeat, col_dt)
    if key not in _CACHE:
        _CACHE[key] = _build_nc(repeat, col_dt)
    return _CACHE[key]


def prepare_in_maps(
    anon_e_emb, e_table, c_table, r_table, fc0_w, fc0_b, c_id, r_id, col_dt="f32"
):
    import ml_dtypes
    e_all = np.ascontiguousarray(
        np.concatenate(
            [np.asarray(e_table, np.float32), np.asarray(anon_e_emb, np.float32)], 0
        )
    )
    fc0_w = np.asarray(fc0_w, np.float32)
    w_l = fc0_w[0, :D]
    b = np.float32(np.asarray(fc0_b, np.float32)[0])
    c_emb = np.asarray(c_table, np.float32)[int(c_id)]
    r_emb = np.asarray(r_table, np.float32)[int(r_id)]
    rw = np.float32(np.dot(r_emb, w_l))
    cw = np.float32(np.dot(c_emb, w_l))

    wb = np.ascontiguousarray(np.broadcast_to(fc0_w[0], (P, 2 * D))).astype(np.float32)
    consts = np.empty((P, 2), np.float32)
    consts[:, 0] = rw + b
    consts[:, 1] = cw + b

    if col_dt == "f32":
        e_col = e_all
        wr_col = wb
        e_row_arr = e_all
    else:
        ndt = ml_dtypes.bfloat16 if col_dt == "bf16" else np.float16
        e_col = np.ascontiguousarray(e_all.astype(ndt))
        wr_col = np.ascontiguousarray(wb.astype(ndt))
        e_row_arr = e_col if ROW_DT_SAME else e_all

    in_maps = []
    for c in range(NCORES):
        in_maps.append(
            {
                "e_full": e_col,
                "wr_c": wr_col,
                "e_rows": np.ascontiguousarray(e_row_arr[c * RPC : (c + 1) * RPC]),
                "wb": wb,
                "consts": consts,
            }
        )
    return in_maps


def run(inputs, trace=False, trace_kwargs=None, repeat=1, col_dt=COL_DT):
    from concourse.bass_utils import run_bass_kernel_spmd

    nc = get_nc(repeat, col_dt)
    in_maps = prepare_in_maps(**inputs, col_dt=col_dt)
    res = run_bass_kernel_spmd(
        nc,
        in_maps,
        core_ids=list(range(NCORES)),
        trace=trace,
        **(trace_kwargs or {}),
    )
    out = np.concatenate([res.results[c]["out"] for c in range(NCORES)])
    return out, res


def kernel(**inputs) -> np.ndarray:
    out, _ = run(inputs, trace=False)
    return out



# revision 5
# speedup vs baseline: 2.1763x; 2.1763x over previous
"""FALCON ObjectSomeValuesFrom forward kernel for Trainium2 (Bass/Tile).

Math: the reference computes
    c_fs[j]   = sigmoid(cw + col_j + b)
    r_fs[i,j] = sigmoid(row_i + col_j + b)
    out[i]    = max_j r_fs[i,j] * c_fs[j]
with col_j = e_j . w_r, row_i = e_i . w_l + rw, cw = c_emb . w_l,
rw = r_emb . w_l.  Both product factors are strictly increasing in col_j,
so the max over j is attained at argmax_j col_j for every i:
    out[i] = sigmoid(a_i + rw + colmax + b) * sigmoid(cw + colmax + b)
with a_i = e_i . w_l and colmax = max_j col_j.  The O(N^2) pairwise block
collapses to two GEMVs over e_all plus an elementwise sigmoid tail.

Implementation: the e-table is transposed on the host to eT [128, 8192]
(k on partitions) and stored in fp8-e4m3 with a power-of-two scale S on
both e and w (products carry S^2, folded into the sigmoid's scale
factor).  Each 128-column block of eT is a natural PE stationary
[K=k, M=128 rows]; rhs = [w_r, w_l] [K=k, N=2] gives out[128 rows, 2] =
both GEMVs per chunk.  64 matmuls fill PSUM [128, 64, 2]; a single DVE
reduce_max + a GPSIMD partition all-reduce produce colmax, then one
sigmoid over the 64 a-columns and a scalar multiply finish the job.

Every core runs the identical program over the full table (the problem
is latency-dominated; a row-shard would not shorten the critical path,
which is one serial ~1MB DMA plus fixed DMA/semaphore latencies).  The
host gathers row-slice c from core c's output.
"""

import numpy as np

N = 8192        # 8000 named + 192 anon entities
D = 128         # emb dim == contraction == partitions
P = 128
NCORES = 8
RPC = N // NCORES     # rows per core (1024)
NCHUNK = N // P       # 64 chunks of 128 rows
COL_DT = "fp8e4"      # "fp8e4" | "fp8e3" | "fp16" | "bf16"
SCALE = {"fp8e4": 8.0, "fp8e3": 32.0, "fp16": 1.0, "bf16": 1.0}
NDMA = 1              # dma_starts for the e-table

_CACHE = {}


def _np_dt(col_dt):
    import ml_dtypes
    return {
        "fp8e4": ml_dtypes.float8_e4m3,
        "fp8e3": ml_dtypes.float8_e3m4,
        "fp16": np.float16,
        "bf16": ml_dtypes.bfloat16,
    }[col_dt]


def _build_nc(repeat=1, col_dt=COL_DT):
    import concourse.bass as bass
    import concourse.bacc as bacc
    import concourse.tile as tile
    import concourse.mybir as mybir
    from concourse import bass_isa

    f32 = mybir.dt.float32
    cdt = {
        "fp8e4": mybir.dt.float8e4,
        "fp8e3": mybir.dt.float8e3,
        "fp16": mybir.dt.float16,
        "bf16": mybir.dt.bfloat16,
    }[col_dt]
    inv_s2 = 1.0 / (SCALE[col_dt] * SCALE[col_dt])
    nc = bacc.Bacc("TRN2", target_bir_lowering=False, debug=False)

    # eT (scaled, transposed) with [w_r, w_l] appended as two extra columns.
    et_d = nc.dram_tensor("et", [P, N + 2], cdt, kind="ExternalInput").ap()
    consts_d = nc.dram_tensor("consts", [P, 2], f32, kind="ExternalInput").ap()
    out_d = nc.dram_tensor("out", [N], f32, kind="ExternalOutput").ap()

    with tile.TileContext(nc) as tc:
        with (
            tc.tile_pool(name="sb", bufs=1) as sb,
            tc.tile_pool(name="ps", bufs=1, space="PSUM") as ps,
        ):
            et = sb.tile([P, N + 2], cdt)
            step = N // NDMA
            for d in range(NDMA):
                nc.sync.dma_start(
                    et[:, d * step : (d + 1) * step],
                    et_d[:, d * step : (d + 1) * step],
                )
            nc.sync.dma_start(et[:, N : N + 2], et_d[:, N : N + 2])
            consts_t = sb.tile([P, 2], f32)
            nc.sync.dma_start(consts_t[:], consts_d)

            # Dependency-free dummy sigmoid: hoists the 1.3us activation
            # table load into the DMA window instead of the critical tail.
            dum = sb.tile([P, 1], f32)
            nc.gpsimd.memset(dum[:], 0.0)
            dum2 = sb.tile([P, 1], f32)
            nc.scalar.activation(
                dum2[:], dum[:], mybir.ActivationFunctionType.Sigmoid
            )

            w2 = et[:, N : N + 2]
            pst = ps.tile([P, NCHUNK * 2], f32)
            psv = pst[:].rearrange("p (n two) -> p n two", two=2)
            for r in range(repeat):
                for c in range(NCHUNK):
                    nc.tensor.matmul(
                        psv[:, c, :],
                        et[:, c * P : (c + 1) * P],
                        w2,
                        start=True,
                        stop=True,
                    )

            # colmax = max over all 8192 col dots (still carrying S^2).
            colm = sb.tile([P, 1], f32)
            nc.vector.reduce_max(colm[:], psv[:, :, 0], axis=mybir.AxisListType.X)
            colmax = sb.tile([P, 1], f32)
            nc.gpsimd.partition_all_reduce(
                colmax[:], colm[:], channels=P, reduce_op=bass_isa.ReduceOp.max
            )

            # k1 = colmax/S^2 + (rw + b);  k2 = sigmoid(colmax/S^2 + (cw + b))
            k1 = sb.tile([P, 1], f32)
            nc.vector.tensor_scalar(
                k1[:], colmax[:], inv_s2, consts_t[:, 0:1],
                op0=mybir.AluOpType.mult, op1=mybir.AluOpType.add,
            )
            k2 = sb.tile([P, 1], f32)
            nc.scalar.activation(
                k2[:], colmax[:], mybir.ActivationFunctionType.Sigmoid,
                bias=consts_t[:, 1:2], scale=inv_s2,
            )

            # out[t*128 + p] = sigmoid(a/S^2 + k1) * k2
            so = sb.tile([P, NCHUNK], f32)
            nc.scalar.activation(
                so[:], psv[:, :, 1], mybir.ActivationFunctionType.Sigmoid,
                bias=k1[:, 0:1], scale=inv_s2,
            )
            fo = sb.tile([P, NCHUNK], f32)
            nc.vector.tensor_scalar_mul(fo[:], so[:], k2[:, 0:1])

            # Host permutes eT columns so PSUM row (p, c) holds entity
            # 64*p + c; the store is then per-partition contiguous (256B
            # runs) instead of a 4-byte-descriptor scatter.
            outv = out_d.rearrange("(p n) -> p n", p=P)
            nc.sync.dma_start(outv, fo[:])

    nc.compile()
    return nc


def get_nc(repeat=1, col_dt=COL_DT):
    key = ("nc", repeat, col_dt)
    if key not in _CACHE:
        _CACHE[key] = _build_nc(repeat, col_dt)
    return _CACHE[key]


def prepare_in_maps(
    anon_e_emb, e_table, c_table, r_table, fc0_w, fc0_b, c_id, r_id, col_dt=COL_DT
):
    e_all = np.concatenate(
        [np.asarray(e_table, np.float32), np.asarray(anon_e_emb, np.float32)], 0
    )  # [N, D]
    fc0_w = np.asarray(fc0_w, np.float32)
    w_l = fc0_w[0, :D]
    w_r = fc0_w[0, D:]
    b = np.float32(np.asarray(fc0_b, np.float32)[0])
    c_emb = np.asarray(c_table, np.float32)[int(c_id)]
    r_emb = np.asarray(r_table, np.float32)[int(r_id)]
    rw = np.float32(np.dot(r_emb, w_l))
    cw = np.float32(np.dot(c_emb, w_l))

    s = SCALE[col_dt]
    ndt = _np_dt(col_dt)
    # Column permutation: device position 128*c + p (chunk c, out partition
    # p) holds entity 64*p + c, so the PSUM result [p, c] maps to the
    # contiguous "(p n)" DRAM layout for the output store.
    perm = (64 * np.arange(P)[None, :] + np.arange(NCHUNK)[:, None]).reshape(-1)
    aug = np.empty((P, N + 2), np.float32)
    aug[:, :N] = e_all.T[:, perm] * s
    aug[:, N] = w_r * s
    aug[:, N + 1] = w_l * s
    aug = np.ascontiguousarray(aug.astype(ndt))

    consts = np.empty((P, 2), np.float32)
    consts[:, 0] = rw + b
    consts[:, 1] = cw + b

    in_map = {"et": aug, "consts": consts}
    return [in_map] * NCORES


def run(inputs, trace=False, trace_kwargs=None, repeat=1, col_dt=COL_DT):
    from concourse.bass_utils import run_bass_kernel_spmd

    nc = get_nc(repeat, col_dt)
    in_maps = prepare_in_maps(**inputs, col_dt=col_dt)
    res = run_bass_kernel_spmd(
        nc,
        in_maps,
        core_ids=list(range(NCORES)),
        trace=trace,
        **(trace_kwargs or {}),
    )
    out = np.concatenate(
        [res.results[c]["out"][c * RPC : (c + 1) * RPC] for c in range(NCORES)]
    )
    return out, res


def kernel(**inputs) -> np.ndarray:
    out, _ = run(inputs, trace=False)
    return out


# revision 13
# speedup vs baseline: 2.2771x; 1.0463x over previous
"""FALCON ObjectSomeValuesFrom forward kernel for Trainium2 (Bass/Tile).

Math: the reference computes
    c_fs[j]   = sigmoid(cw + col_j + b)
    r_fs[i,j] = sigmoid(row_i + col_j + b)
    out[i]    = max_j r_fs[i,j] * c_fs[j]
with col_j = e_j . w_r, row_i = e_i . w_l + rw, cw = c_emb . w_l,
rw = r_emb . w_l.  Both product factors are strictly increasing in col_j,
so the max over j is attained at argmax_j col_j for every i:
    out[i] = sigmoid(a_i + rw + colmax + b) * sigmoid(cw + colmax + b)
with a_i = e_i . w_l and colmax = max_j col_j.  The O(N^2) pairwise block
collapses to two GEMVs over e_all plus an elementwise sigmoid tail.

Implementation: the e-table is transposed on the host to eT [128, 8192]
(k on partitions) and stored in fp8-e4m3 with a power-of-two scale S on
both e and w (products carry S^2, folded into the sigmoid's scale
factor).  Each 128-column block of eT is a natural PE stationary
[K=k, M=128 rows]; rhs = [w_r, w_l] [K=k, N=2] gives out[128 rows, 2] =
both GEMVs per chunk.  64 matmuls fill PSUM [128, 64, 2]; DVE
reduce_max + a GPSIMD partition all-reduce produce colmax, then one
sigmoid over the 64 a-columns and a scalar multiply finish the job.
The e-table DMA is split so most matmuls and a partial reduce overlap
the final slice's transfer, and the output store is a pre-generated
SWDGE scatter fired by trigger_dma (prep descriptors are written during
the main DMA window; only the trigger + transfer sit on the tail).

Every core runs the identical program over the full table (the problem
is latency-dominated; a row-shard would not shorten the critical path,
which is one serial ~1MB DMA plus fixed DMA/semaphore latencies).  The
host gathers row-slice c from core c's output.
"""

import numpy as np

N = 8192        # 8000 named + 192 anon entities
D = 128         # emb dim == contraction == partitions
P = 128
NCORES = 8
RPC = N // NCORES     # rows per core (1024)
NCHUNK = N // P       # 64 chunks of 128 rows
COL_DT = "fp8e4"      # "fp8e4" | "fp8e3" | "fp16" | "bf16"
SCALE = {"fp8e4": 8.0, "fp8e3": 32.0, "fp16": 1.0, "bf16": 1.0}
SPLIT = 56            # chunks in the first e-DMA (0/64 = single DMA)
# Output via prepped SWDGE scatter + trigger_dma. NOTE: functionally
# correct (CoreSim-verified) but TimelineSim's no_exec mode cannot model
# InstIncSwdgeSem's executor-applied sem pre-bumps, so the timing
# simulator deadlocks on any gen_mode==1 prep — keep False.
SCATTER_OUT = False

_CACHE = {}


def _np_dt(col_dt):
    import ml_dtypes
    return {
        "fp8e4": ml_dtypes.float8_e4m3,
        "fp8e3": ml_dtypes.float8_e3m4,
        "fp16": np.float16,
        "bf16": ml_dtypes.bfloat16,
    }[col_dt]


def _build_nc(repeat=1, col_dt=COL_DT):
    import concourse.bass as bass
    import concourse.bacc as bacc
    import concourse.tile as tile
    import concourse.mybir as mybir
    from concourse import bass_isa

    f32 = mybir.dt.float32
    cdt = {
        "fp8e4": mybir.dt.float8e4,
        "fp8e3": mybir.dt.float8e3,
        "fp16": mybir.dt.float16,
        "bf16": mybir.dt.bfloat16,
    }[col_dt]
    inv_s2 = 1.0 / (SCALE[col_dt] * SCALE[col_dt])
    nc = bacc.Bacc("TRN2", target_bir_lowering=False, debug=False)

    # [w_r, w_l] in cols 0:2, then the scaled/transposed/permuted e-table.
    et_d = nc.dram_tensor("et", [P, N + 2], cdt, kind="ExternalInput").ap()
    consts_d = nc.dram_tensor("consts", [P, 2], f32, kind="ExternalInput").ap()
    if SCATTER_OUT:
        idx_d = nc.dram_tensor("idx", [P, NCHUNK // 8], mybir.dt.int16,
                               kind="ExternalInput").ap()
    f16 = mybir.dt.float16
    out_d = nc.dram_tensor("out", [N], f16, kind="ExternalOutput").ap()

    split = SPLIT if 0 < SPLIT < NCHUNK else NCHUNK

    with tile.TileContext(nc) as tc:
        with (
            tc.tile_pool(name="sb", bufs=1) as sb,
            tc.tile_pool(name="ps", bufs=1, space="PSUM") as ps,
        ):
            et = sb.tile([P, N + 2], cdt)
            cut = 2 + split * P
            nc.sync.dma_start(et[:, 0:cut], et_d[:, 0:cut])
            if cut < N + 2:
                nc.sync.dma_start(et[:, cut:], et_d[:, cut:])
            consts_t = sb.tile([P, 2], f32)
            nc.sync.dma_start(consts_t[:], consts_d)
            if SCATTER_OUT:
                idx_t = sb.tile([P, NCHUNK // 8], mybir.dt.int16)
                nc.sync.dma_start(idx_t[:], idx_d)

            # Dependency-free dummy sigmoid: hoists the 1.3us activation
            # table load into the DMA window instead of the critical tail.
            dum = sb.tile([P, 1], f32)
            nc.vector.memset(dum[:], 0.0)
            dum2 = sb.tile([P, 1], f32)
            nc.scalar.activation(
                dum2[:], dum[:], mybir.ActivationFunctionType.Sigmoid
            )

            w2 = et[:, 0:2]
            pst = ps.tile([P, NCHUNK * 2], f32)
            psv = pst[:].rearrange("p (n two) -> p n two", two=2)
            for r in range(repeat):
                for c in range(NCHUNK):
                    nc.tensor.matmul(
                        psv[:, c, :],
                        et[:, 2 + c * P : 2 + (c + 1) * P],
                        w2,
                        start=True,
                        stop=True,
                    )

            # colmax = max over all 8192 col dots (still carrying S^2).
            colm = sb.tile([P, 1], f32)
            nc.vector.reduce_max(colm[:], psv[:, :, 0], axis=mybir.AxisListType.X)
            # Fold the affine map into the per-partition value BEFORE the
            # partition all-reduce (max commutes with x/S^2 + c0), so the
            # Pool output is directly the sigmoid bias k1 — one hop fewer.
            k1p = sb.tile([P, 1], f32)
            nc.vector.tensor_scalar(
                k1p[:], colm[:], inv_s2, consts_t[:, 0:1],
                op0=mybir.AluOpType.mult, op1=mybir.AluOpType.add,
            )
            k1 = sb.tile([P, 1], f32)
            nc.gpsimd.partition_all_reduce(
                k1[:], k1p[:], channels=P, reduce_op=bass_isa.ReduceOp.max
            )
            # k2 = sigmoid(colmax/S^2 + cw + b) = sigmoid(k1 + (cw - rw))
            k2 = sb.tile([P, 1], f32)
            nc.scalar.activation(
                k2[:], k1[:], mybir.ActivationFunctionType.Sigmoid,
                bias=consts_t[:, 1:2], scale=1.0,
            )

            # out[p*64 + c] = sigmoid(a/S^2 + k1) * k2
            so = sb.tile([P, NCHUNK], f16)
            nc.scalar.activation(
                so[:], psv[:, :, 1], mybir.ActivationFunctionType.Sigmoid,
                bias=k1[:, 0:1], scale=inv_s2,
            )
            fo = sb.tile([P, NCHUNK], f16)
            nc.vector.tensor_scalar_mul(fo[:], so[:], k2[:, 0:1])

            if SCATTER_OUT:
                # Descriptors are generated during the DMA window (prep only
                # reads idx_t); the trigger carries the RAW dep on fo.
                dma_sem = nc.alloc_semaphore("out_dma")
                nc.gpsimd.dma_scatter_add(
                    out_d.rearrange("(t e) -> t e", e=NCHUNK),
                    fo[:].rearrange("p (t e) -> p t e", t=1),
                    idx_t[:],
                    P,            # num_idxs: 128 tokens of 64 floats
                    P,
                    NCHUNK,       # elem_size (64 f32 = 256B)
                    prepare_only=True,
                    sem=dma_sem,
                )
                nc.gpsimd.trigger_dma(count=None)
            else:
                outv = out_d.rearrange("(p n) -> p n", p=P)
                nc.sync.dma_start(outv, fo[:])

    nc.compile()
    return nc


def get_nc(repeat=1, col_dt=COL_DT):
    key = ("nc", repeat, col_dt)
    if key not in _CACHE:
        _CACHE[key] = _build_nc(repeat, col_dt)
    return _CACHE[key]


def prepare_in_maps(
    anon_e_emb, e_table, c_table, r_table, fc0_w, fc0_b, c_id, r_id, col_dt=COL_DT
):
    e_all = np.concatenate(
        [np.asarray(e_table, np.float32), np.asarray(anon_e_emb, np.float32)], 0
    )  # [N, D]
    fc0_w = np.asarray(fc0_w, np.float32)
    w_l = fc0_w[0, :D]
    w_r = fc0_w[0, D:]
    b = np.float32(np.asarray(fc0_b, np.float32)[0])
    c_emb = np.asarray(c_table, np.float32)[int(c_id)]
    r_emb = np.asarray(r_table, np.float32)[int(r_id)]
    rw = np.float32(np.dot(r_emb, w_l))
    cw = np.float32(np.dot(c_emb, w_l))

    s = SCALE[col_dt]
    ndt = _np_dt(col_dt)
    # Column permutation: device position 128*c + p (chunk c, out partition
    # p) holds entity 64*p + c, so the PSUM result [p, c] maps to the
    # contiguous "(p n)" DRAM layout for the output store.
    perm = (64 * np.arange(P)[None, :] + np.arange(NCHUNK)[:, None]).reshape(-1)
    aug = np.empty((P, N + 2), np.float32)
    aug[:, 0] = w_r * s
    aug[:, 1] = w_l * s
    aug[:, 2:] = e_all.T[:, perm] * s
    aug = np.ascontiguousarray(aug.astype(ndt))

    consts = np.empty((P, 2), np.float32)
    consts[:, 0] = rw + b
    consts[:, 1] = cw - rw  # k2 bias on top of k1 = colmax/S^2 + rw + b

    in_map = {"et": aug, "consts": consts}
    if SCATTER_OUT:
        # Token i's index lives at idx[i % 16, i // 16] (only the first 16
        # partitions are read; the rest is padding). Identity scatter.
        idx16 = np.arange(P, dtype=np.int16).reshape(P // 16, 16).T  # [16, 8]
        idx = np.tile(idx16, (P // 16, 1))
        in_map["idx"] = np.ascontiguousarray(idx)
    return [in_map] * NCORES


def run(inputs, trace=False, trace_kwargs=None, repeat=1, col_dt=COL_DT):
    from concourse.bass_utils import run_bass_kernel_spmd

    nc = get_nc(repeat, col_dt)
    in_maps = prepare_in_maps(**inputs, col_dt=col_dt)
    res = run_bass_kernel_spmd(
        nc,
        in_maps,
        core_ids=list(range(NCORES)),
        trace=trace,
        **(trace_kwargs or {}),
    )
    out = np.concatenate(
        [res.results[c]["out"][c * RPC : (c + 1) * RPC] for c in range(NCORES)]
    ).astype(np.float32)
    return out, res


def kernel(**inputs) -> np.ndarray:
    out, _ = run(inputs, trace=False)
    return out
